# revision 52
# baseline (speedup 1.0000x reference)
"""DVQ bottleneck kernel for Trainium2, data-parallel over 8 NeuronCores.

Problem (hardcoded): h [8, 4096, 1024] f32, codebooks [4, 16, 256] f32.
Per token t and slice s: ids[t,s] = argmin_k ||ze_ts - c_sk||^2,
z = gathered codebook rows, ids packed base-16, vq loss = 1.25 * sum of
min squared distances / (B*N*d).

Sharding: 32768 tokens split 4096/core across 8 cores; codebooks replicated.

h is re-encoded on the host as an exact-to-2^-17 bf16 hi/lo pair
(h = a1 + a2) so the device can use the 2-byte DMA xbar transpose and
bf16 matmuls throughout; no fp32 tensor ever streams through the PE.

Per-core dataflow:
  DMA-xbar-transpose hbf tile straight from DRAM -> a12T [dp, 16, t]
  (chunks 0..7 = a1T, 8..15 = a2T) -> PE matmul with stationary
  [c_hi | c_lo] codebook chunks -> scoresT [s*32, t] PSUM -> copy +
  PE-transpose back to token-major -> DVE: fold halves + |c|^2,
  reduce_min, first-index argmin via is_equal/iota trick -> one-hot
  duplicated x2 (bf16) -> PE transpose -> one matmul per 512-col half
  against [c_hi ; c_lo] stacked in the contraction dim -> z [t, 1024]
  -> copy -> DMA out.
  Loss: hT = a1T + a2T on GPSIMD, ACT square+accum, min-scores on DVE,
  final partition reduce via ones-matmul.
"""

import sys
import types

import numpy as np
import ml_dtypes

# If the image lacks antenv.axon_hooks, trace=True/BASS_TRACE paths in
# bass_utils would crash on import; register a no-op hook registry instead.
try:
    from antenv import axon_hooks as _ah  # noqa: F401
except Exception:
    try:
        import antenv

        _mod = types.ModuleType("antenv.axon_hooks")
        _mod._HOOK = None
        _mod.set_axon_ntff_profile_hook = lambda h: setattr(_mod, "_HOOK", h)
        _mod.get_axon_ntff_profile_hook = lambda: _mod._HOOK
        sys.modules["antenv.axon_hooks"] = _mod
        antenv.axon_hooks = _mod
    except Exception:
        pass

import concourse.bass as bass
import concourse.bacc as bacc
import concourse.mybir as mybir
from concourse.tile import TileContext
from concourse.bass_utils import run_bass_kernel_spmd
from concourse.masks import make_identity

F32 = mybir.dt.float32
BF16 = mybir.dt.bfloat16
I32 = mybir.dt.int32

B, N, D = 8, 4096, 1024
S, K, d = 4, 16, 256
NCORES = 8
P = 128
NCHUNK = D // P  # 8
BETA = 0.25


def build_nc(t_core: int) -> bass.Bass:
    nsub = t_core // P
    nmac = nsub // 4
    assert nmac * 4 == nsub

    nc = bacc.Bacc()

    hbf_d = nc.declare_dram_parameter(
        "hbf", [t_core // (4 * P), P, 2 * NCHUNK, 4 * P], BF16, isOutput=False
    )
    ctsb_d = nc.declare_dram_parameter("ctsb", [P, NCHUNK, 2 * K], BF16, isOutput=False)
    cbd12_d = nc.declare_dram_parameter("cbd12", [2 * S * K, D], BF16, isOutput=False)
    cnorm_d = nc.declare_dram_parameter("cnorm4", [P, 4 * S * K], F32, isOutput=False)
    iotad_d = nc.declare_dram_parameter("iotad8", [P, 2 * 4 * S * K], F32, isOutput=False)
    offs_d = nc.declare_dram_parameter("offs4", [P, 4 * S], F32, isOutput=False)
    z_d = nc.declare_dram_parameter("z", [t_core, D], F32, isOutput=True)
    ids_d = nc.declare_dram_parameter("ids", [nsub, P], I32, isOutput=True)
    part_d = nc.declare_dram_parameter("partial", [1, 1], F32, isOutput=True)

    with TileContext(nc) as tc:
        with (
            tc.tile_pool(name="consts", bufs=1) as consts,
            tc.tile_pool(name="atp", bufs=5) as atp,
            tc.tile_pool(name="zsb", bufs=2) as zsb_pool,
            tc.tile_pool(name="small", bufs=3) as small,
            tc.tile_pool(name="accp", bufs=1) as accp,
            tc.tile_pool(name="ps_scT", bufs=2, space="PSUM") as ps_scT,
            tc.tile_pool(name="ps_raw", bufs=2, space="PSUM") as ps_raw,
            tc.tile_pool(name="ps_oh", bufs=1, space="PSUM") as ps_oh,
            tc.tile_pool(name="ps_z", bufs=3, space="PSUM") as ps_z,
        ):
            # ---- constants ----
            identity = consts.tile([P, P], F32)
            make_identity(nc, identity)
            identity_bf = consts.tile([P, P], BF16)
            nc.vector.tensor_copy(out=identity_bf, in_=identity)
            ctsb = consts.tile([P, NCHUNK, 2 * K], BF16)
            nc.sync.dma_start(out=ctsb, in_=ctsb_d[:, :, :])
            cbd12 = consts.tile([2 * S * K, D], BF16)
            nc.sync.dma_start(out=cbd12, in_=cbd12_d[:, :])
            cnorm4 = consts.tile([P, 4 * S * K], F32)
            nc.sync.dma_start(out=cnorm4, in_=cnorm_d[:, :])
            iotad8 = consts.tile([P, 2 * 4 * S * K], F32)
            nc.sync.dma_start(out=iotad8, in_=iotad_d[:, :])
            offs4 = consts.tile([P, 4 * S], F32)
            nc.sync.dma_start(out=offs4, in_=offs_d[:, :])
            ones = consts.tile([P, 1], F32)
            nc.vector.memset(ones, 1.0)

            # ---- accumulators ----
            acc16 = accp.tile([P, 4 * S], F32)
            nc.vector.memset(acc16, 0.0)
            idsf = accp.tile([P, nsub], F32)

            pending_stores: list[tuple] = []
            for m in range(nmac):
                t0m = 4 * m * P
                # the host stores hbf pre-transposed: one plain full-rate load
                # gives a12T[dp, c, t] with c 0..7 -> a1T, c 8..15 -> a2T
                a12T = atp.tile([P, 2 * NCHUNK, 4 * P], BF16, tag="a12T")
                nc.sync.dma_start(out=a12T, in_=hbf_d[m])

                # z stores ride the ACT DGE ring, independent of the SP ring
                # that carries the transposes
                while len(pending_stores) > 1:
                    dst, src = pending_stores.pop(0)
                    nc.scalar.dma_start(out=dst, in_=src)

                scT_ps = ps_scT.tile([P, 4, P], F32, tag="scT")
                # scoresT = (-2C)^T ze per slice: contract d in 2 chunks,
                # hi/lo codebook columns; the a2T pass accumulates the lo
                # part of ze. All 512 macro tokens stream in one matmul.
                for s in range(S):
                    for gi, (cc, plane) in enumerate(
                        ((2 * s, 0), (2 * s, 1), (2 * s + 1, 0), (2 * s + 1, 1))
                    ):
                        nc.tensor.matmul(
                            scT_ps[32 * s : 32 * s + 32, :, :],
                            lhsT=ctsb[:, cc, :],
                            rhs=a12T[:, plane * NCHUNK + cc, :],
                            start=(gi == 0),
                            stop=(gi == 3),
                            tile_position=(0, 32 * s),
                        )



                # ---- back to token-major: copy + PE transpose ----
                scT_sb = small.tile([P, 4, P], F32, tag="scTsb")
                nc.scalar.copy(out=scT_sb, in_=scT_ps)
                sc_raw = ps_raw.tile([P, 4, S, 2, K], F32, tag="raw")
                for j in range(4):
                    nc.tensor.transpose(
                        sc_raw[:, j], scT_sb[:, j, :], identity
                    )
                # fold hi/lo halves via a size-2 innermost reduce, then |c|^2
                sc_sb = small.tile([P, 4, S, K], F32, tag="scsb")
                nc.vector.tensor_reduce(
                    sc_sb,
                    sc_raw.rearrange("p j s u k -> p (j s) k u"),
                    axis=mybir.AxisListType.X,
                    op=mybir.AluOpType.add,
                )
                sc2 = small.tile([P, 4 * S * K], F32, tag="sc2")
                nc.vector.tensor_tensor(
                    sc2, sc_sb.rearrange("p a s k -> p (a s k)"), cnorm4,
                    mybir.AluOpType.add,
                )
                sc3 = sc2.rearrange("p (a s k) -> p a s k", s=S, k=K)
                minv = small.tile([P, 4 * S], F32, tag="minv")
                nc.vector.tensor_reduce(
                    minv,
                    sc2.rearrange("p (g k) -> p g k", k=K),
                    axis=mybir.AxisListType.X,
                    op=mybir.AluOpType.min,
                )
                mask = small.tile([P, 4 * S * K], F32, tag="mask")
                nc.vector.tensor_tensor(
                    mask,
                    sc3,
                    minv.rearrange("p (a s) -> p a s", s=S)[:, :, :, None]
                        .to_broadcast((P, 4, S, K)),
                    mybir.AluOpType.is_equal,
                )
                t4 = small.tile([P, 4 * S * K], F32, tag="t4")
                nc.vector.tensor_tensor(
                    t4, mask, iotad8[:, : 4 * S * K], mybir.AluOpType.mult
                )
                dmax = small.tile([P, 4 * S], F32, tag="dmax")
                nc.vector.tensor_reduce(
                    dmax,
                    t4.rearrange("p (g k) -> p g k", k=K),
                    axis=mybir.AxisListType.X,
                    op=mybir.AluOpType.max,
                )
                # packed ids: 65535 - sum_s dmax * 16^s   (dmax = 15 - id)
                pk = small.tile([P, 4 * S], F32, tag="pk")
                nc.vector.tensor_tensor(pk, dmax, offs4, mybir.AluOpType.mult)
                pneg = small.tile([P, 4], F32, tag="pneg")
                nc.vector.tensor_reduce(
                    pneg,
                    pk.rearrange("p (a s) -> p a s", s=S),
                    axis=mybir.AxisListType.X,
                    op=mybir.AluOpType.add,
                )
                nc.vector.tensor_scalar(
                    idsf[:, 4 * m : 4 * m + 4], pneg, -1.0, 65535.0,
                    op0=mybir.AluOpType.mult, op1=mybir.AluOpType.add,
                )
                # clean one-hot (single 1 even on ties), duplicated x2 so the
                # z matmul can stack the hi/lo codebooks in the contraction:
                # onehotT2[t, (j, u, s, k)] = (iotad == dmax[j, s]), u = 0, 1
                onehotT2 = small.tile([P, 4, 2, S, K], BF16, tag="onehotT2")
                for u in range(2):
                    nc.vector.tensor_tensor(
                        onehotT2[:, :, u],
                        iotad8[:, : 4 * S * K].rearrange(
                            "p (a s k) -> p a s k", s=S, k=K
                        ),
                        dmax.rearrange("p (a s) -> p a s", s=S)[:, :, :, None]
                            .to_broadcast((P, 4, S, K)),
                        mybir.AluOpType.is_equal,
                    )
                # loss accumulation
                nc.vector.tensor_tensor(acc16, acc16, minv, mybir.AluOpType.add)

                # ---- gather z via one-hot matmul (bf16, exact to 2^-17) ----
                oh_ps = ps_oh.tile([2 * S * K, 4, P], BF16, tag="oh")
                for j in range(4):
                    nc.tensor.transpose(
                        oh_ps[:, j, :],
                        onehotT2[:, j],
                        identity_bf,
                    )
                oh_sb = small.tile([2 * S * K, 4, P], BF16, tag="ohsb")
                nc.scalar.copy(out=oh_sb, in_=oh_ps)
                z_mac = zsb_pool.tile([P, 4, D], F32, tag="zs")
                for j in range(4):
                    for half in (0, 1):
                        sl = slice(half * (D // 2), (half + 1) * (D // 2))
                        z_ps = ps_z.tile([P, D // 2], F32, tag="z")
                        nc.tensor.matmul(
                            z_ps, lhsT=oh_sb[:, j, :], rhs=cbd12[:, sl],
                            start=True, stop=True,
                        )
                        if (2 * j + half) % 2 == 0:
                            nc.scalar.copy(out=z_mac[:, j, sl], in_=z_ps)
                        else:
                            nc.vector.tensor_copy(out=z_mac[:, j, sl], in_=z_ps)
                pending_stores.append(
                    (
                        z_d[t0m : t0m + 4 * P, :].rearrange(
                            "(j p) dd -> p j dd", p=P
                        ),
                        z_mac,
                    )
                )

            for dst, src in pending_stores:
                nc.scalar.dma_start(out=dst, in_=src)
            pending_stores.clear()

            # ---- epilogue: ids out ----
            ids_ps = ps_raw.tile([nsub, P], F32, tag="raw")
            nc.tensor.transpose(ids_ps, idsf, identity)
            ids_int = small.tile([nsub, P], I32, tag="idsint")
            nc.vector.tensor_copy(out=ids_int, in_=ids_ps)
            nc.sync.dma_start(out=ids_d[:, :], in_=ids_int)

            # ---- epilogue: loss partial ----
            rt = small.tile([P, 1], F32, tag="rt")
            nc.vector.tensor_reduce(
                rt, acc16, axis=mybir.AxisListType.X, op=mybir.AluOpType.add
            )
            part_ps = ps_oh.tile([1, 1], F32, tag="oh")
            nc.tensor.matmul(part_ps, lhsT=ones, rhs=rt, start=True, stop=True)
            part_sb = small.tile([1, 1], F32, tag="partsb")
            nc.vector.tensor_copy(out=part_sb, in_=part_ps)
            nc.sync.dma_start(out=part_d[:, :], in_=part_sb)

    nc.finalize()
    return nc


def make_consts(cb: np.ndarray) -> dict[str, np.ndarray]:
    cb = np.ascontiguousarray(cb, dtype=np.float32)
    # cts[dp, c, k] = -2 * cb[c // 2, k, (c % 2) * 128 + dp], split hi/lo bf16
    cb_r = cb.reshape(S, K, 2, P)  # [s, k, half, dp]
    cts = np.ascontiguousarray(-2.0 * cb_r.transpose(3, 0, 2, 1).reshape(P, NCHUNK, K))
    cts1 = cts.astype(ml_dtypes.bfloat16)
    cts2 = (cts - cts1.astype(np.float32)).astype(ml_dtypes.bfloat16)
    ctsb = np.concatenate([cts1, cts2], axis=-1)  # [P, NCHUNK, 2K]
    cbd = np.zeros((S * K, D), dtype=np.float32)
    for s in range(S):
        cbd[s * K : (s + 1) * K, s * d : (s + 1) * d] = cb[s]
    cbd1 = cbd.astype(ml_dtypes.bfloat16)
    cbd2 = (cbd - cbd1.astype(np.float32)).astype(ml_dtypes.bfloat16)
    cbd12 = np.concatenate([cbd1, cbd2], axis=0)  # [2*S*K, D]
    cnorm1 = np.square(cb).sum(-1).reshape(-1)  # [S*K]
    cnorm4 = np.tile(cnorm1, (P, 4))
    iotad1 = np.tile((15.0 - np.arange(K)).astype(np.float32), 4 * S)
    iotad8 = np.tile(iotad1, (P, 2))
    offs1 = np.tile((16.0 ** np.arange(S)).astype(np.float32), 4)
    offs4 = np.tile(offs1, (P, 1))
    return {
        "ctsb": np.ascontiguousarray(ctsb),
        "cbd12": np.ascontiguousarray(cbd12),
        "cnorm4": cnorm4.astype(np.float32),
        "iotad8": iotad8.astype(np.float32),
        "offs4": offs4.astype(np.float32),
    }


def make_hbf(h: np.ndarray) -> np.ndarray:
    """Exact-to-2^-17 bf16 hi/lo re-encoding (h = a1 + a2), stored
    pre-transposed per 512-token macro tile:
    hbf[m, dp, u*8+c, t] = a_u[512m + t, 128c + dp]."""
    hf = np.ascontiguousarray(h, dtype=np.float32).reshape(-1, D)
    a1 = hf.astype(ml_dtypes.bfloat16)
    a2 = (hf - a1.astype(np.float32)).astype(ml_dtypes.bfloat16)
    A = np.stack([a1, a2], axis=1)          # [T, u, d]
    M = hf.shape[0] // (4 * P)
    A = A.reshape(M, 4 * P, 2, NCHUNK, P)   # [m, t, u, c, dp]
    X = A.transpose(0, 4, 2, 3, 1)          # [m, dp, u, c, t]
    return np.ascontiguousarray(X.reshape(M, P, 2 * NCHUNK, 4 * P))


_NC_CACHE: dict[int, bass.Bass] = {}
LAST_RESULTS = None


def _get_nc(t_core: int) -> bass.Bass:
    if t_core not in _NC_CACHE:
        _NC_CACHE[t_core] = build_nc(t_core)
    return _NC_CACHE[t_core]


def kernel(h: np.ndarray, codebooks: np.ndarray):
    h = np.ascontiguousarray(h, dtype=np.float32)
    cb = np.ascontiguousarray(codebooks, dtype=np.float32)
    t_core = (B * N) // NCORES
    nc = _get_nc(t_core)
    consts = make_consts(cb)
    hbf = make_hbf(h)
    hf64 = h.reshape(B * N, D)
    mac_core = t_core // (4 * P)
    in_maps = []
    for c in range(NCORES):
        m = {"hbf": hbf[c * mac_core : (c + 1) * mac_core]}
        m.update(consts)
        in_maps.append(m)
    global LAST_RESULTS
    kr = run_bass_kernel_spmd(nc, in_maps, list(range(NCORES)))
    LAST_RESULTS = kr
    res = kr.results
    z = np.concatenate([res[c]["z"] for c in range(NCORES)], axis=0).reshape(B, N, D)
    ids = np.concatenate(
        [res[c]["ids"].reshape(-1) for c in range(NCORES)]
    ).reshape(B, N).astype(np.int32)
    total = np.sum([np.float64(res[c]["partial"][0, 0]) for c in range(NCORES)])
    hsq = np.einsum("ij,ij->", hf64, hf64)
    vq_total = np.float32((1.0 + BETA) * (total + hsq) / (B * N * d))
    return z, ids, vq_total


# revision 54
# speedup vs baseline: 1.0989x; 1.0989x over previous
"""DVQ bottleneck kernel for Trainium2, data-parallel over 8 NeuronCores.

Problem (hardcoded): h [8, 4096, 1024] f32, codebooks [4, 16, 256] f32.
Per token t and slice s: ids[t,s] = argmin_k ||ze_ts - c_sk||^2,
z = gathered codebook rows, ids packed base-16, vq loss = 1.25 * sum of
min squared distances / (B*N*d).

Sharding: 32768 tokens split 4096/core across 8 cores; codebooks replicated.

h is re-encoded on the host as an exact-to-2^-17 bf16 hi/lo pair
(h = a1 + a2) so the device can use the 2-byte DMA xbar transpose and
bf16 matmuls throughout; no fp32 tensor ever streams through the PE.

Per-core dataflow:
  DMA-xbar-transpose hbf tile straight from DRAM -> a12T [dp, 16, t]
  (chunks 0..7 = a1T, 8..15 = a2T) -> PE matmul with stationary
  [c_hi | c_lo] codebook chunks -> scoresT [s*32, t] PSUM -> copy +
  PE-transpose back to token-major -> DVE: fold halves + |c|^2,
  reduce_min, first-index argmin via is_equal/iota trick -> one-hot
  duplicated x2 (bf16) -> PE transpose -> one matmul per 512-col half
  against [c_hi ; c_lo] stacked in the contraction dim -> z [t, 1024]
  -> copy -> DMA out.
  Loss: hT = a1T + a2T on GPSIMD, ACT square+accum, min-scores on DVE,
  final partition reduce via ones-matmul.
"""

import sys
import types

import numpy as np
import ml_dtypes

# If the image lacks antenv.axon_hooks, trace=True/BASS_TRACE paths in
# bass_utils would crash on import; register a no-op hook registry instead.
try:
    from antenv import axon_hooks as _ah  # noqa: F401
except Exception:
    try:
        import antenv

        _mod = types.ModuleType("antenv.axon_hooks")
        _mod._HOOK = None
        _mod.set_axon_ntff_profile_hook = lambda h: setattr(_mod, "_HOOK", h)
        _mod.get_axon_ntff_profile_hook = lambda: _mod._HOOK
        sys.modules["antenv.axon_hooks"] = _mod
        antenv.axon_hooks = _mod
    except Exception:
        pass

import concourse.bass as bass
import concourse.bacc as bacc
import concourse.mybir as mybir
from concourse.tile import TileContext
from concourse.bass_utils import run_bass_kernel_spmd
from concourse.masks import make_identity

F32 = mybir.dt.float32
BF16 = mybir.dt.bfloat16
I32 = mybir.dt.int32

B, N, D = 8, 4096, 1024
S, K, d = 4, 16, 256
NCORES = 8
P = 128
NCHUNK = D // P  # 8
BETA = 0.25


def build_nc(t_core: int) -> bass.Bass:
    nsub = t_core // P
    nmac = nsub // 4
    assert nmac * 4 == nsub

    nc = bacc.Bacc()

    hbf_d = nc.declare_dram_parameter(
        "hbf", [t_core // (4 * P), P, 2 * NCHUNK, 4 * P], BF16, isOutput=False
    )
    ctsb_d = nc.declare_dram_parameter("ctsb", [P, NCHUNK, 2 * K], BF16, isOutput=False)
    cbd12_d = nc.declare_dram_parameter("cbd12", [2 * S * K, D], BF16, isOutput=False)
    cnorm_d = nc.declare_dram_parameter("cnorm4", [P, 4 * S * K], F32, isOutput=False)
    iotad_d = nc.declare_dram_parameter("iotad8", [P, 2 * 4 * S * K], F32, isOutput=False)
    offs_d = nc.declare_dram_parameter("offs4", [P, 4 * S], F32, isOutput=False)
    z_d = nc.declare_dram_parameter("z", [t_core, D], F32, isOutput=True)
    ids_d = nc.declare_dram_parameter("ids", [nsub, P], I32, isOutput=True)
    part_d = nc.declare_dram_parameter("partial", [1, 1], F32, isOutput=True)

    with TileContext(nc) as tc:
        with (
            tc.tile_pool(name="consts", bufs=1) as consts,
            tc.tile_pool(name="atp", bufs=4) as atp,
            tc.tile_pool(name="zsb", bufs=3) as zsb_pool,
            tc.tile_pool(name="small", bufs=3) as small,
            tc.tile_pool(name="accp", bufs=1) as accp,
            tc.tile_pool(name="ps_scT", bufs=2, space="PSUM") as ps_scT,
            tc.tile_pool(name="ps_raw", bufs=2, space="PSUM") as ps_raw,
            tc.tile_pool(name="ps_oh", bufs=1, space="PSUM") as ps_oh,
            tc.tile_pool(name="ps_z", bufs=3, space="PSUM") as ps_z,
        ):
            # ---- constants ----
            identity = consts.tile([P, P], F32)
            make_identity(nc, identity)
            identity_bf = consts.tile([P, P], BF16)
            nc.vector.tensor_copy(out=identity_bf, in_=identity)
            ctsb = consts.tile([P, NCHUNK, 2 * K], BF16)
            nc.sync.dma_start(out=ctsb, in_=ctsb_d[:, :, :])
            cbd12 = consts.tile([2 * S * K, D], BF16)
            nc.sync.dma_start(out=cbd12, in_=cbd12_d[:, :])
            cnorm4 = consts.tile([P, 4 * S * K], F32)
            nc.sync.dma_start(out=cnorm4, in_=cnorm_d[:, :])
            iotad8 = consts.tile([P, 2 * 4 * S * K], F32)
            nc.sync.dma_start(out=iotad8, in_=iotad_d[:, :])
            offs4 = consts.tile([P, 4 * S], F32)
            nc.sync.dma_start(out=offs4, in_=offs_d[:, :])
            ones = consts.tile([P, 1], F32)
            nc.vector.memset(ones, 1.0)

            # ---- accumulators ----
            acc16 = accp.tile([P, 4 * S], F32)
            nc.vector.memset(acc16, 0.0)
            idsf = accp.tile([P, nsub], F32)

            pending_stores: list[tuple] = []
            for m in range(nmac):
                t0m = 4 * m * P
                # the host stores hbf pre-transposed: one plain full-rate load
                # gives a12T[dp, c, t] with c 0..7 -> a1T, c 8..15 -> a2T
                a12T = atp.tile([P, 2 * NCHUNK, 4 * P], BF16, tag="a12T")
                nc.sync.dma_start(out=a12T, in_=hbf_d[m])

                # z stores ride the ACT DGE ring, independent of the SP ring
                # that carries the transposes
                while len(pending_stores) > 1:
                    dst, src = pending_stores.pop(0)
                    nc.scalar.dma_start(out=dst, in_=src)

                scT_ps = ps_scT.tile([P, 4, P], F32, tag="scT")
                # scoresT = (-2C)^T ze per slice: contract d in 2 chunks,
                # hi/lo codebook columns; the a2T pass accumulates the lo
                # part of ze. All 512 macro tokens stream in one matmul.
                for s in range(S):
                    for gi, (cc, plane) in enumerate(
                        ((2 * s, 0), (2 * s, 1), (2 * s + 1, 0), (2 * s + 1, 1))
                    ):
                        nc.tensor.matmul(
                            scT_ps[32 * s : 32 * s + 32, :, :],
                            lhsT=ctsb[:, cc, :],
                            rhs=a12T[:, plane * NCHUNK + cc, :],
                            start=(gi == 0),
                            stop=(gi == 3),
                            tile_position=(0, 32 * s),
                        )



                # ---- back to token-major: copy + PE transpose ----
                scT_sb = small.tile([P, 4, P], F32, tag="scTsb")
                nc.scalar.copy(out=scT_sb, in_=scT_ps)
                sc_raw = ps_raw.tile([P, 4, S, 2, K], F32, tag="raw")
                for j in range(4):
                    nc.tensor.transpose(
                        sc_raw[:, j], scT_sb[:, j, :], identity
                    )
                # fold hi/lo halves via a size-2 innermost reduce, then |c|^2
                sc_sb = small.tile([P, 4, S, K], F32, tag="scsb")
                nc.vector.tensor_reduce(
                    sc_sb,
                    sc_raw.rearrange("p j s u k -> p (j s) k u"),
                    axis=mybir.AxisListType.X,
                    op=mybir.AluOpType.add,
                )
                sc2 = small.tile([P, 4 * S * K], F32, tag="sc2")
                nc.vector.tensor_tensor(
                    sc2, sc_sb.rearrange("p a s k -> p (a s k)"), cnorm4,
                    mybir.AluOpType.add,
                )
                sc3 = sc2.rearrange("p (a s k) -> p a s k", s=S, k=K)
                minv = small.tile([P, 4 * S], F32, tag="minv")
                nc.vector.tensor_reduce(
                    minv,
                    sc2.rearrange("p (g k) -> p g k", k=K),
                    axis=mybir.AxisListType.X,
                    op=mybir.AluOpType.min,
                )
                mask = small.tile([P, 4 * S * K], F32, tag="mask")
                nc.vector.tensor_tensor(
                    mask,
                    sc3,
                    minv.rearrange("p (a s) -> p a s", s=S)[:, :, :, None]
                        .to_broadcast((P, 4, S, K)),
                    mybir.AluOpType.is_equal,
                )
                t4 = small.tile([P, 4 * S * K], F32, tag="t4")
                nc.vector.tensor_tensor(
                    t4, mask, iotad8[:, : 4 * S * K], mybir.AluOpType.mult
                )
                dmax = small.tile([P, 4 * S], F32, tag="dmax")
                nc.vector.tensor_reduce(
                    dmax,
                    t4.rearrange("p (g k) -> p g k", k=K),
                    axis=mybir.AxisListType.X,
                    op=mybir.AluOpType.max,
                )
                # packed ids: 65535 - sum_s dmax * 16^s   (dmax = 15 - id)
                pk = small.tile([P, 4 * S], F32, tag="pk")
                nc.vector.tensor_tensor(pk, dmax, offs4, mybir.AluOpType.mult)
                pneg = small.tile([P, 4], F32, tag="pneg")
                nc.vector.tensor_reduce(
                    pneg,
                    pk.rearrange("p (a s) -> p a s", s=S),
                    axis=mybir.AxisListType.X,
                    op=mybir.AluOpType.add,
                )
                nc.vector.tensor_scalar(
                    idsf[:, 4 * m : 4 * m + 4], pneg, -1.0, 65535.0,
                    op0=mybir.AluOpType.mult, op1=mybir.AluOpType.add,
                )
                # clean one-hot (single 1 even on ties), duplicated x2 so the
                # z matmul can stack the hi/lo codebooks in the contraction:
                # onehotT2[t, (j, u, s, k)] = (iotad == dmax[j, s]), u = 0, 1
                onehotT2 = small.tile([P, 4, 2, S, K], BF16, tag="onehotT2")
                for u in range(2):
                    nc.vector.tensor_tensor(
                        onehotT2[:, :, u],
                        iotad8[:, : 4 * S * K].rearrange(
                            "p (a s k) -> p a s k", s=S, k=K
                        ),
                        dmax.rearrange("p (a s) -> p a s", s=S)[:, :, :, None]
                            .to_broadcast((P, 4, S, K)),
                        mybir.AluOpType.is_equal,
                    )
                # loss accumulation
                nc.vector.tensor_tensor(acc16, acc16, minv, mybir.AluOpType.add)

                # ---- gather z via one-hot matmul (bf16, exact to 2^-17) ----
                oh_ps = ps_oh.tile([2 * S * K, 4, P], BF16, tag="oh")
                for j in range(4):
                    nc.tensor.transpose(
                        oh_ps[:, j, :],
                        onehotT2[:, j],
                        identity_bf,
                    )
                oh_sb = small.tile([2 * S * K, 4, P], BF16, tag="ohsb")
                nc.scalar.copy(out=oh_sb, in_=oh_ps)
                z_mac = zsb_pool.tile([P, 4, D], F32, tag="zs")
                for j in range(4):
                    for half in (0, 1):
                        sl = slice(half * (D // 2), (half + 1) * (D // 2))
                        z_ps = ps_z.tile([P, D // 2], F32, tag="z")
                        nc.tensor.matmul(
                            z_ps, lhsT=oh_sb[:, j, :], rhs=cbd12[:, sl],
                            start=True, stop=True,
                        )
                        if (2 * j + half) % 4 != 3:
                            nc.scalar.copy(out=z_mac[:, j, sl], in_=z_ps)
                        else:
                            nc.vector.tensor_copy(out=z_mac[:, j, sl], in_=z_ps)
                pending_stores.append(
                    (
                        z_d[t0m : t0m + 4 * P, :].rearrange(
                            "(j p) dd -> p j dd", p=P
                        ),
                        z_mac,
                    )
                )

            for dst, src in pending_stores:
                nc.scalar.dma_start(out=dst, in_=src)
            pending_stores.clear()

            # ---- epilogue: ids out ----
            ids_ps = ps_raw.tile([nsub, P], F32, tag="raw")
            nc.tensor.transpose(ids_ps, idsf, identity)
            ids_int = small.tile([nsub, P], I32, tag="idsint")
            nc.vector.tensor_copy(out=ids_int, in_=ids_ps)
            nc.sync.dma_start(out=ids_d[:, :], in_=ids_int)

            # ---- epilogue: loss partial ----
            rt = small.tile([P, 1], F32, tag="rt")
            nc.vector.tensor_reduce(
                rt, acc16, axis=mybir.AxisListType.X, op=mybir.AluOpType.add
            )
            part_ps = ps_oh.tile([1, 1], F32, tag="oh")
            nc.tensor.matmul(part_ps, lhsT=ones, rhs=rt, start=True, stop=True)
            part_sb = small.tile([1, 1], F32, tag="partsb")
            nc.vector.tensor_copy(out=part_sb, in_=part_ps)
            nc.sync.dma_start(out=part_d[:, :], in_=part_sb)

    nc.finalize()
    return nc


def make_consts(cb: np.ndarray) -> dict[str, np.ndarray]:
    cb = np.ascontiguousarray(cb, dtype=np.float32)
    # cts[dp, c, k] = -2 * cb[c // 2, k, (c % 2) * 128 + dp], split hi/lo bf16
    cb_r = cb.reshape(S, K, 2, P)  # [s, k, half, dp]
    cts = np.ascontiguousarray(-2.0 * cb_r.transpose(3, 0, 2, 1).reshape(P, NCHUNK, K))
    cts1 = cts.astype(ml_dtypes.bfloat16)
    cts2 = (cts - cts1.astype(np.float32)).astype(ml_dtypes.bfloat16)
    ctsb = np.concatenate([cts1, cts2], axis=-1)  # [P, NCHUNK, 2K]
    cbd = np.zeros((S * K, D), dtype=np.float32)
    for s in range(S):
        cbd[s * K : (s + 1) * K, s * d : (s + 1) * d] = cb[s]
    cbd1 = cbd.astype(ml_dtypes.bfloat16)
    cbd2 = (cbd - cbd1.astype(np.float32)).astype(ml_dtypes.bfloat16)
    cbd12 = np.concatenate([cbd1, cbd2], axis=0)  # [2*S*K, D]
    cnorm1 = np.square(cb).sum(-1).reshape(-1)  # [S*K]
    cnorm4 = np.tile(cnorm1, (P, 4))
    iotad1 = np.tile((15.0 - np.arange(K)).astype(np.float32), 4 * S)
    iotad8 = np.tile(iotad1, (P, 2))
    offs1 = np.tile((16.0 ** np.arange(S)).astype(np.float32), 4)
    offs4 = np.tile(offs1, (P, 1))
    return {
        "ctsb": np.ascontiguousarray(ctsb),
        "cbd12": np.ascontiguousarray(cbd12),
        "cnorm4": cnorm4.astype(np.float32),
        "iotad8": iotad8.astype(np.float32),
        "offs4": offs4.astype(np.float32),
    }


def make_hbf(h: np.ndarray) -> np.ndarray:
    """Exact-to-2^-17 bf16 hi/lo re-encoding (h = a1 + a2), stored
    pre-transposed per 512-token macro tile:
    hbf[m, dp, u*8+c, t] = a_u[512m + t, 128c + dp]."""
    hf = np.ascontiguousarray(h, dtype=np.float32).reshape(-1, D)
    a1 = hf.astype(ml_dtypes.bfloat16)
    a2 = (hf - a1.astype(np.float32)).astype(ml_dtypes.bfloat16)
    A = np.stack([a1, a2], axis=1)          # [T, u, d]
    M = hf.shape[0] // (4 * P)
    A = A.reshape(M, 4 * P, 2, NCHUNK, P)   # [m, t, u, c, dp]
    X = A.transpose(0, 4, 2, 3, 1)          # [m, dp, u, c, t]
    return np.ascontiguousarray(X.reshape(M, P, 2 * NCHUNK, 4 * P))


_NC_CACHE: dict[int, bass.Bass] = {}
LAST_RESULTS = None


def _get_nc(t_core: int) -> bass.Bass:
    if t_core not in _NC_CACHE:
        _NC_CACHE[t_core] = build_nc(t_core)
    return _NC_CACHE[t_core]


def kernel(h: np.ndarray, codebooks: np.ndarray):
    h = np.ascontiguousarray(h, dtype=np.float32)
    cb = np.ascontiguousarray(codebooks, dtype=np.float32)
    t_core = (B * N) // NCORES
    nc = _get_nc(t_core)
    consts = make_consts(cb)
    hbf = make_hbf(h)
    hf64 = h.reshape(B * N, D)
    mac_core = t_core // (4 * P)
    in_maps = []
    for c in range(NCORES):
        m = {"hbf": hbf[c * mac_core : (c + 1) * mac_core]}
        m.update(consts)
        in_maps.append(m)
    global LAST_RESULTS
    kr = run_bass_kernel_spmd(nc, in_maps, list(range(NCORES)))
    LAST_RESULTS = kr
    res = kr.results
    z = np.concatenate([res[c]["z"] for c in range(NCORES)], axis=0).reshape(B, N, D)
    ids = np.concatenate(
        [res[c]["ids"].reshape(-1) for c in range(NCORES)]
    ).reshape(B, N).astype(np.int32)
    total = np.sum([np.float64(res[c]["partial"][0, 0]) for c in range(NCORES)])
    hsq = np.einsum("ij,ij->", hf64, hf64)
    vq_total = np.float32((1.0 + BETA) * (total + hsq) / (B * N * d))
    return z, ids, vq_total


# revision 55
# speedup vs baseline: 1.1033x; 1.0041x over previous
"""DVQ bottleneck kernel for Trainium2, data-parallel over 8 NeuronCores.

Problem (hardcoded): h [8, 4096, 1024] f32, codebooks [4, 16, 256] f32.
Per token t and slice s: ids[t,s] = argmin_k ||ze_ts - c_sk||^2,
z = gathered codebook rows, ids packed base-16, vq loss = 1.25 * sum of
min squared distances / (B*N*d).

Sharding: 32768 tokens split 4096/core across 8 cores; codebooks replicated.

h is re-encoded on the host as an exact-to-2^-17 bf16 hi/lo pair
(h = a1 + a2) so the device can use the 2-byte DMA xbar transpose and
bf16 matmuls throughout; no fp32 tensor ever streams through the PE.

Per-core dataflow:
  DMA-xbar-transpose hbf tile straight from DRAM -> a12T [dp, 16, t]
  (chunks 0..7 = a1T, 8..15 = a2T) -> PE matmul with stationary
  [c_hi | c_lo] codebook chunks -> scoresT [s*32, t] PSUM -> copy +
  PE-transpose back to token-major -> DVE: fold halves + |c|^2,
  reduce_min, first-index argmin via is_equal/iota trick -> one-hot
  duplicated x2 (bf16) -> PE transpose -> one matmul per 512-col half
  against [c_hi ; c_lo] stacked in the contraction dim -> z [t, 1024]
  -> copy -> DMA out.
  Loss: hT = a1T + a2T on GPSIMD, ACT square+accum, min-scores on DVE,
  final partition reduce via ones-matmul.
"""

import sys
import types

import numpy as np
import ml_dtypes

# If the image lacks antenv.axon_hooks, trace=True/BASS_TRACE paths in
# bass_utils would crash on import; register a no-op hook registry instead.
try:
    from antenv import axon_hooks as _ah  # noqa: F401
except Exception:
    try:
        import antenv

        _mod = types.ModuleType("antenv.axon_hooks")
        _mod._HOOK = None
        _mod.set_axon_ntff_profile_hook = lambda h: setattr(_mod, "_HOOK", h)
        _mod.get_axon_ntff_profile_hook = lambda: _mod._HOOK
        sys.modules["antenv.axon_hooks"] = _mod
        antenv.axon_hooks = _mod
    except Exception:
        pass

import concourse.bass as bass
import concourse.bacc as bacc
import concourse.mybir as mybir
from concourse.tile import TileContext
from concourse.bass_utils import run_bass_kernel_spmd
from concourse.masks import make_identity

F32 = mybir.dt.float32
BF16 = mybir.dt.bfloat16
I32 = mybir.dt.int32

B, N, D = 8, 4096, 1024
S, K, d = 4, 16, 256
NCORES = 8
P = 128
NCHUNK = D // P  # 8
BETA = 0.25


def build_nc(t_core: int) -> bass.Bass:
    nsub = t_core // P
    nmac = nsub // 4
    assert nmac * 4 == nsub

    nc = bacc.Bacc()

    hbf_d = nc.declare_dram_parameter(
        "hbf", [t_core // (4 * P), P, 2 * NCHUNK, 4 * P], BF16, isOutput=False
    )
    ctsb_d = nc.declare_dram_parameter("ctsb", [P, NCHUNK, 2 * K], BF16, isOutput=False)
    cbd12_d = nc.declare_dram_parameter("cbd12", [2 * S * K, D], BF16, isOutput=False)
    cnorm_d = nc.declare_dram_parameter("cnorm4", [P, 4 * S * K], F32, isOutput=False)
    iotad_d = nc.declare_dram_parameter("iotad8", [P, 2 * 4 * S * K], F32, isOutput=False)
    offs_d = nc.declare_dram_parameter("offs4", [P, 4 * S], F32, isOutput=False)
    z_d = nc.declare_dram_parameter("z", [t_core, D], F32, isOutput=True)
    ids_d = nc.declare_dram_parameter("ids", [nsub, P], I32, isOutput=True)
    part_d = nc.declare_dram_parameter("partial", [1, 1], F32, isOutput=True)

    with TileContext(nc) as tc:
        with (
            tc.tile_pool(name="consts", bufs=1) as consts,
            tc.tile_pool(name="atp", bufs=5) as atp,
            tc.tile_pool(name="zsb", bufs=3) as zsb_pool,
            tc.tile_pool(name="small", bufs=3) as small,
            tc.tile_pool(name="accp", bufs=1) as accp,
            tc.tile_pool(name="ps_scT", bufs=2, space="PSUM") as ps_scT,
            tc.tile_pool(name="ps_raw", bufs=2, space="PSUM") as ps_raw,
            tc.tile_pool(name="ps_oh", bufs=1, space="PSUM") as ps_oh,
            tc.tile_pool(name="ps_z", bufs=3, space="PSUM") as ps_z,
        ):
            # ---- constants ----
            identity = consts.tile([P, P], F32)
            make_identity(nc, identity)
            identity_bf = consts.tile([P, P], BF16)
            nc.vector.tensor_copy(out=identity_bf, in_=identity)
            ctsb = consts.tile([P, NCHUNK, 2 * K], BF16)
            nc.sync.dma_start(out=ctsb, in_=ctsb_d[:, :, :])
            cbd12 = consts.tile([2 * S * K, D], BF16)
            nc.sync.dma_start(out=cbd12, in_=cbd12_d[:, :])
            cnorm4 = consts.tile([P, 4 * S * K], F32)
            nc.sync.dma_start(out=cnorm4, in_=cnorm_d[:, :])
            iotad8 = consts.tile([P, 2 * 4 * S * K], F32)
            nc.sync.dma_start(out=iotad8, in_=iotad_d[:, :])
            offs4 = consts.tile([P, 4 * S], F32)
            nc.sync.dma_start(out=offs4, in_=offs_d[:, :])
            ones = consts.tile([P, 1], F32)
            nc.vector.memset(ones, 1.0)

            # ---- accumulators ----
            acc16 = accp.tile([P, 4 * S], F32)
            nc.vector.memset(acc16, 0.0)
            idsf = accp.tile([P, nsub], F32)

            pending_stores: list[tuple] = []
            for m in range(nmac):
                t0m = 4 * m * P
                # the host stores hbf pre-transposed: one plain full-rate load
                # gives a12T[dp, c, t] with c 0..7 -> a1T, c 8..15 -> a2T
                a12T = atp.tile([P, 2 * NCHUNK, 4 * P], BF16, tag="a12T")
                nc.sync.dma_start(out=a12T, in_=hbf_d[m])

                # z stores ride the ACT DGE ring, independent of the SP ring
                # that carries the transposes
                while len(pending_stores) > 1:
                    dst, src = pending_stores.pop(0)
                    nc.scalar.dma_start(out=dst, in_=src)

                scT_ps = ps_scT.tile([P, 4, P], F32, tag="scT")
                # scoresT = (-2C)^T ze per slice: contract d in 2 chunks,
                # hi/lo codebook columns; the a2T pass accumulates the lo
                # part of ze. All 512 macro tokens stream in one matmul.
                for s in range(S):
                    for gi, (cc, plane) in enumerate(
                        ((2 * s, 0), (2 * s, 1), (2 * s + 1, 0), (2 * s + 1, 1))
                    ):
                        nc.tensor.matmul(
                            scT_ps[32 * s : 32 * s + 32, :, :],
                            lhsT=ctsb[:, cc, :],
                            rhs=a12T[:, plane * NCHUNK + cc, :],
                            start=(gi == 0),
                            stop=(gi == 3),
                            tile_position=(0, 32 * s),
                        )



                # ---- back to token-major: copy + PE transpose ----
                scT_sb = small.tile([P, 4, P], F32, tag="scTsb")
                nc.scalar.copy(out=scT_sb, in_=scT_ps)
                sc_raw = ps_raw.tile([P, 4, S, 2, K], F32, tag="raw")
                for j in range(4):
                    nc.tensor.transpose(
                        sc_raw[:, j], scT_sb[:, j, :], identity
                    )
                # fold hi/lo halves via a size-2 innermost reduce, then |c|^2
                sc_sb = small.tile([P, 4, S, K], F32, tag="scsb")
                nc.vector.tensor_reduce(
                    sc_sb,
                    sc_raw.rearrange("p j s u k -> p (j s) k u"),
                    axis=mybir.AxisListType.X,
                    op=mybir.AluOpType.add,
                )
                sc2 = small.tile([P, 4 * S * K], F32, tag="sc2")
                nc.vector.tensor_tensor(
                    sc2, sc_sb.rearrange("p a s k -> p (a s k)"), cnorm4,
                    mybir.AluOpType.add,
                )
                sc3 = sc2.rearrange("p (a s k) -> p a s k", s=S, k=K)
                minv = small.tile([P, 4 * S], F32, tag="minv")
                nc.vector.tensor_reduce(
                    minv,
                    sc2.rearrange("p (g k) -> p g k", k=K),
                    axis=mybir.AxisListType.X,
                    op=mybir.AluOpType.min,
                )
                mask = small.tile([P, 4 * S * K], F32, tag="mask")
                nc.vector.tensor_tensor(
                    mask,
                    sc3,
                    minv.rearrange("p (a s) -> p a s", s=S)[:, :, :, None]
                        .to_broadcast((P, 4, S, K)),
                    mybir.AluOpType.is_equal,
                )
                t4 = small.tile([P, 4 * S * K], F32, tag="t4")
                nc.vector.tensor_tensor(
                    t4, mask, iotad8[:, : 4 * S * K], mybir.AluOpType.mult
                )
                dmax = small.tile([P, 4 * S], F32, tag="dmax")
                nc.vector.tensor_reduce(
                    dmax,
                    t4.rearrange("p (g k) -> p g k", k=K),
                    axis=mybir.AxisListType.X,
                    op=mybir.AluOpType.max,
                )
                # packed ids: 65535 - sum_s dmax * 16^s   (dmax = 15 - id)
                pk = small.tile([P, 4 * S], F32, tag="pk")
                nc.vector.tensor_tensor(pk, dmax, offs4, mybir.AluOpType.mult)
                pneg = small.tile([P, 4], F32, tag="pneg")
                nc.vector.tensor_reduce(
                    pneg,
                    pk.rearrange("p (a s) -> p a s", s=S),
                    axis=mybir.AxisListType.X,
                    op=mybir.AluOpType.add,
                )
                nc.vector.tensor_scalar(
                    idsf[:, 4 * m : 4 * m + 4], pneg, -1.0, 65535.0,
                    op0=mybir.AluOpType.mult, op1=mybir.AluOpType.add,
                )
                # clean one-hot (single 1 even on ties), duplicated x2 so the
                # z matmul can stack the hi/lo codebooks in the contraction:
                # onehotT2[t, (j, u, s, k)] = (iotad == dmax[j, s]), u = 0, 1
                onehotT2 = small.tile([P, 4, 2, S, K], BF16, tag="onehotT2")
                for u in range(2):
                    nc.vector.tensor_tensor(
                        onehotT2[:, :, u],
                        iotad8[:, : 4 * S * K].rearrange(
                            "p (a s k) -> p a s k", s=S, k=K
                        ),
                        dmax.rearrange("p (a s) -> p a s", s=S)[:, :, :, None]
                            .to_broadcast((P, 4, S, K)),
                        mybir.AluOpType.is_equal,
                    )
                # loss accumulation
                nc.vector.tensor_tensor(acc16, acc16, minv, mybir.AluOpType.add)

                # ---- gather z via one-hot matmul (bf16, exact to 2^-17) ----
                oh_ps = ps_oh.tile([2 * S * K, 4, P], BF16, tag="oh")
                for j in range(4):
                    nc.tensor.transpose(
                        oh_ps[:, j, :],
                        onehotT2[:, j],
                        identity_bf,
                    )
                oh_sb = small.tile([2 * S * K, 4, P], BF16, tag="ohsb")
                nc.scalar.copy(out=oh_sb, in_=oh_ps)
                z_mac = zsb_pool.tile([P, 4, D], F32, tag="zs")
                for j in range(4):
                    for half in (0, 1):
                        sl = slice(half * (D // 2), (half + 1) * (D // 2))
                        z_ps = ps_z.tile([P, D // 2], F32, tag="z")
                        nc.tensor.matmul(
                            z_ps, lhsT=oh_sb[:, j, :], rhs=cbd12[:, sl],
                            start=True, stop=True,
                        )
                        if (2 * j + half) % 4 != 3:
                            nc.scalar.copy(out=z_mac[:, j, sl], in_=z_ps)
                        else:
                            nc.vector.tensor_copy(out=z_mac[:, j, sl], in_=z_ps)
                pending_stores.append(
                    (
                        z_d[t0m : t0m + 4 * P, :].rearrange(
                            "(j p) dd -> p j dd", p=P
                        ),
                        z_mac,
                    )
                )

            for dst, src in pending_stores:
                nc.scalar.dma_start(out=dst, in_=src)
            pending_stores.clear()

            # ---- epilogue: ids out ----
            ids_ps = ps_raw.tile([nsub, P], F32, tag="raw")
            nc.tensor.transpose(ids_ps, idsf, identity)
            ids_int = small.tile([nsub, P], I32, tag="idsint")
            nc.vector.tensor_copy(out=ids_int, in_=ids_ps)
            nc.sync.dma_start(out=ids_d[:, :], in_=ids_int)

            # ---- epilogue: loss partial ----
            rt = small.tile([P, 1], F32, tag="rt")
            nc.vector.tensor_reduce(
                rt, acc16, axis=mybir.AxisListType.X, op=mybir.AluOpType.add
            )
            part_ps = ps_oh.tile([1, 1], F32, tag="oh")
            nc.tensor.matmul(part_ps, lhsT=ones, rhs=rt, start=True, stop=True)
            part_sb = small.tile([1, 1], F32, tag="partsb")
            nc.vector.tensor_copy(out=part_sb, in_=part_ps)
            nc.sync.dma_start(out=part_d[:, :], in_=part_sb)

    nc.finalize()
    return nc


def make_consts(cb: np.ndarray) -> dict[str, np.ndarray]:
    cb = np.ascontiguousarray(cb, dtype=np.float32)
    # cts[dp, c, k] = -2 * cb[c // 2, k, (c % 2) * 128 + dp], split hi/lo bf16
    cb_r = cb.reshape(S, K, 2, P)  # [s, k, half, dp]
    cts = np.ascontiguousarray(-2.0 * cb_r.transpose(3, 0, 2, 1).reshape(P, NCHUNK, K))
    cts1 = cts.astype(ml_dtypes.bfloat16)
    cts2 = (cts - cts1.astype(np.float32)).astype(ml_dtypes.bfloat16)
    ctsb = np.concatenate([cts1, cts2], axis=-1)  # [P, NCHUNK, 2K]
    cbd = np.zeros((S * K, D), dtype=np.float32)
    for s in range(S):
        cbd[s * K : (s + 1) * K, s * d : (s + 1) * d] = cb[s]
    cbd1 = cbd.astype(ml_dtypes.bfloat16)
    cbd2 = (cbd - cbd1.astype(np.float32)).astype(ml_dtypes.bfloat16)
    cbd12 = np.concatenate([cbd1, cbd2], axis=0)  # [2*S*K, D]
    cnorm1 = np.square(cb).sum(-1).reshape(-1)  # [S*K]
    cnorm4 = np.tile(cnorm1, (P, 4))
    iotad1 = np.tile((15.0 - np.arange(K)).astype(np.float32), 4 * S)
    iotad8 = np.tile(iotad1, (P, 2))
    offs1 = np.tile((16.0 ** np.arange(S)).astype(np.float32), 4)
    offs4 = np.tile(offs1, (P, 1))
    return {
        "ctsb": np.ascontiguousarray(ctsb),
        "cbd12": np.ascontiguousarray(cbd12),
        "cnorm4": cnorm4.astype(np.float32),
        "iotad8": iotad8.astype(np.float32),
        "offs4": offs4.astype(np.float32),
    }


def make_hbf(h: np.ndarray) -> np.ndarray:
    """Exact-to-2^-17 bf16 hi/lo re-encoding (h = a1 + a2), stored
    pre-transposed per 512-token macro tile:
    hbf[m, dp, u*8+c, t] = a_u[512m + t, 128c + dp]."""
    hf = np.ascontiguousarray(h, dtype=np.float32).reshape(-1, D)
    a1 = hf.astype(ml_dtypes.bfloat16)
    a2 = (hf - a1.astype(np.float32)).astype(ml_dtypes.bfloat16)
    A = np.stack([a1, a2], axis=1)          # [T, u, d]
    M = hf.shape[0] // (4 * P)
    A = A.reshape(M, 4 * P, 2, NCHUNK, P)   # [m, t, u, c, dp]
    X = A.transpose(0, 4, 2, 3, 1)          # [m, dp, u, c, t]
    return np.ascontiguousarray(X.reshape(M, P, 2 * NCHUNK, 4 * P))


_NC_CACHE: dict[int, bass.Bass] = {}
LAST_RESULTS = None


def _get_nc(t_core: int) -> bass.Bass:
    if t_core not in _NC_CACHE:
        _NC_CACHE[t_core] = build_nc(t_core)
    return _NC_CACHE[t_core]


def kernel(h: np.ndarray, codebooks: np.ndarray):
    h = np.ascontiguousarray(h, dtype=np.float32)
    cb = np.ascontiguousarray(codebooks, dtype=np.float32)
    t_core = (B * N) // NCORES
    nc = _get_nc(t_core)
    consts = make_consts(cb)
    hbf = make_hbf(h)
    hf64 = h.reshape(B * N, D)
    mac_core = t_core // (4 * P)
    in_maps = []
    for c in range(NCORES):
        m = {"hbf": hbf[c * mac_core : (c + 1) * mac_core]}
        m.update(consts)
        in_maps.append(m)
    global LAST_RESULTS
    kr = run_bass_kernel_spmd(nc, in_maps, list(range(NCORES)))
    LAST_RESULTS = kr
    res = kr.results
    z = np.concatenate([res[c]["z"] for c in range(NCORES)], axis=0).reshape(B, N, D)
    ids = np.concatenate(
        [res[c]["ids"].reshape(-1) for c in range(NCORES)]
    ).reshape(B, N).astype(np.int32)
    total = np.sum([np.float64(res[c]["partial"][0, 0]) for c in range(NCORES)])
    hsq = np.einsum("ij,ij->", hf64, hf64)
    vq_total = np.float32((1.0 + BETA) * (total + hsq) / (B * N * d))
    return z, ids, vq_total


# revision 58
# speedup vs baseline: 1.2434x; 1.1270x over previous
"""DVQ bottleneck kernel for Trainium2, data-parallel over 8 NeuronCores.

Problem (hardcoded): h [8, 4096, 1024] f32, codebooks [4, 16, 256] f32.
Per token t and slice s: ids[t,s] = argmin_k ||ze_ts - c_sk||^2,
z = gathered codebook rows, ids packed base-16, vq loss = 1.25 * sum of
min squared distances / (B*N*d).

Sharding: 32768 tokens split 4096/core across 8 cores; codebooks replicated.

h is re-encoded on the host as an exact-to-2^-17 bf16 hi/lo pair
(h = a1 + a2) so the device can use the 2-byte DMA xbar transpose and
bf16 matmuls throughout; no fp32 tensor ever streams through the PE.

Per-core dataflow:
  DMA-xbar-transpose hbf tile straight from DRAM -> a12T [dp, 16, t]
  (chunks 0..7 = a1T, 8..15 = a2T) -> PE matmul with stationary
  [c_hi | c_lo] codebook chunks -> scoresT [s*32, t] PSUM -> copy +
  PE-transpose back to token-major -> DVE: fold halves + |c|^2,
  reduce_min, first-index argmin via is_equal/iota trick -> one-hot
  duplicated x2 (bf16) -> PE transpose -> one matmul per 512-col half
  against [c_hi ; c_lo] stacked in the contraction dim -> z [t, 1024]
  -> copy -> DMA out.
  Loss: hT = a1T + a2T on GPSIMD, ACT square+accum, min-scores on DVE,
  final partition reduce via ones-matmul.
"""

import sys
import types

import numpy as np
import ml_dtypes

# If the image lacks antenv.axon_hooks, trace=True/BASS_TRACE paths in
# bass_utils would crash on import; register a no-op hook registry instead.
try:
    from antenv import axon_hooks as _ah  # noqa: F401
except Exception:
    try:
        import antenv

        _mod = types.ModuleType("antenv.axon_hooks")
        _mod._HOOK = None
        _mod.set_axon_ntff_profile_hook = lambda h: setattr(_mod, "_HOOK", h)
        _mod.get_axon_ntff_profile_hook = lambda: _mod._HOOK
        sys.modules["antenv.axon_hooks"] = _mod
        antenv.axon_hooks = _mod
    except Exception:
        pass

import concourse.bass as bass
import concourse.bacc as bacc
import concourse.mybir as mybir
from concourse.tile import TileContext
from concourse.bass_utils import run_bass_kernel_spmd
from concourse.masks import make_identity

F32 = mybir.dt.float32
BF16 = mybir.dt.bfloat16
I32 = mybir.dt.int32

B, N, D = 8, 4096, 1024
S, K, d = 4, 16, 256
NCORES = 8
P = 128
NCHUNK = D // P  # 8
BETA = 0.25


def build_nc(t_core: int) -> bass.Bass:
    nsub = t_core // P
    nmac = nsub // 4
    assert nmac * 4 == nsub

    nc = bacc.Bacc()

    hbf_d = nc.declare_dram_parameter(
        "hbf", [t_core // (4 * P), P, 2 * NCHUNK, 4 * P], BF16, isOutput=False
    )
    ctsb_d = nc.declare_dram_parameter("ctsb", [P, NCHUNK, 2 * K], BF16, isOutput=False)
    cbd12_d = nc.declare_dram_parameter("cbd12", [2 * S * K, D], BF16, isOutput=False)
    cnorm_d = nc.declare_dram_parameter("cnorm4", [P, 4 * S * K], F32, isOutput=False)
    iotad_d = nc.declare_dram_parameter("iotad8", [P, 2 * 4 * S * K], F32, isOutput=False)
    offs_d = nc.declare_dram_parameter("offs4", [P, 4 * S], F32, isOutput=False)
    z_d = nc.declare_dram_parameter("z", [t_core, D], F32, isOutput=True)
    ids_d = nc.declare_dram_parameter("ids", [nsub, P], I32, isOutput=True)
    part_d = nc.declare_dram_parameter("partial", [1, 1], F32, isOutput=True)

    with TileContext(nc) as tc:
        with (
            tc.tile_pool(name="consts", bufs=1) as consts,
            tc.tile_pool(name="atp", bufs=6) as atp,
            tc.tile_pool(name="zsb", bufs=3) as zsb_pool,
            tc.tile_pool(name="small", bufs=3) as small,
            tc.tile_pool(name="accp", bufs=1) as accp,
            tc.tile_pool(name="ps_scT", bufs=2, space="PSUM") as ps_scT,
            tc.tile_pool(name="ps_raw", bufs=2, space="PSUM") as ps_raw,
            tc.tile_pool(name="ps_oh", bufs=1, space="PSUM") as ps_oh,
            tc.tile_pool(name="ps_z", bufs=3, space="PSUM") as ps_z,
        ):
            # prefetch the first macro tiles ahead of the constant loads so
            # the big streams lead the SP DGE ring
            prefetched: dict[int, object] = {}
            for m in range(min(2, nmac)):
                a12T = atp.tile([P, 2 * NCHUNK, 4 * P], BF16, tag="a12T")
                nc.sync.dma_start(out=a12T, in_=hbf_d[m])
                prefetched[m] = a12T

            # ---- constants ----
            identity = consts.tile([P, P], F32)
            make_identity(nc, identity)
            identity_bf = consts.tile([P, P], BF16)
            nc.vector.tensor_copy(out=identity_bf, in_=identity)
            ctsb = consts.tile([P, NCHUNK, 2 * K], BF16)
            nc.sync.dma_start(out=ctsb, in_=ctsb_d[:, :, :])
            cbd12 = consts.tile([2 * S * K, D], BF16)
            nc.sync.dma_start(out=cbd12, in_=cbd12_d[:, :])
            cnorm4 = consts.tile([P, 4 * S * K], F32)
            nc.sync.dma_start(out=cnorm4, in_=cnorm_d[:, :])
            iotad8 = consts.tile([P, 2 * 4 * S * K], F32)
            nc.sync.dma_start(out=iotad8, in_=iotad_d[:, :])
            offs4 = consts.tile([P, 4 * S], F32)
            nc.sync.dma_start(out=offs4, in_=offs_d[:, :])
            ones = consts.tile([P, 1], F32)
            nc.vector.memset(ones, 1.0)

            # ---- accumulators ----
            acc16 = accp.tile([P, 4 * S], F32)
            nc.vector.memset(acc16, 0.0)
            idsf = accp.tile([P, nsub], F32)

            pending_stores: list[tuple] = []
            for m in range(nmac):
                t0m = 4 * m * P
                # the host stores hbf pre-transposed: one plain full-rate load
                # gives a12T[dp, c, t] with c 0..7 -> a1T, c 8..15 -> a2T
                if m in prefetched:
                    a12T = prefetched.pop(m)
                else:
                    a12T = atp.tile([P, 2 * NCHUNK, 4 * P], BF16, tag="a12T")
                    nc.sync.dma_start(out=a12T, in_=hbf_d[m])

                # z stores ride the ACT DGE ring, independent of the SP ring
                # that carries the transposes
                while len(pending_stores) > 1:
                    dst, src = pending_stores.pop(0)
                    nc.scalar.dma_start(out=dst, in_=src)

                scT_ps = ps_scT.tile([P, 4, P], F32, tag="scT")
                # scoresT = (-2C)^T ze per slice: contract d in 2 chunks,
                # hi/lo codebook columns; the a2T pass accumulates the lo
                # part of ze. All 512 macro tokens stream in one matmul.
                for s in range(S):
                    for gi, (cc, plane) in enumerate(
                        ((2 * s, 0), (2 * s, 1), (2 * s + 1, 0), (2 * s + 1, 1))
                    ):
                        nc.tensor.matmul(
                            scT_ps[32 * s : 32 * s + 32, :, :],
                            lhsT=ctsb[:, cc, :],
                            rhs=a12T[:, plane * NCHUNK + cc, :],
                            start=(gi == 0),
                            stop=(gi == 3),
                            tile_position=(0, 32 * s),
                        )



                # ---- back to token-major: copy + PE transpose ----
                scT_sb = small.tile([P, 4, P], F32, tag="scTsb")
                nc.scalar.copy(out=scT_sb, in_=scT_ps)
                sc_raw = ps_raw.tile([P, 4, S, 2, K], F32, tag="raw")
                for j in range(4):
                    nc.tensor.transpose(
                        sc_raw[:, j], scT_sb[:, j, :], identity
                    )
                # fold hi/lo halves via a size-2 innermost reduce, then |c|^2
                sc_sb = small.tile([P, 4, S, K], F32, tag="scsb")
                nc.vector.tensor_reduce(
                    sc_sb,
                    sc_raw.rearrange("p j s u k -> p (j s) k u"),
                    axis=mybir.AxisListType.X,
                    op=mybir.AluOpType.add,
                )
                sc2 = small.tile([P, 4 * S * K], F32, tag="sc2")
                nc.vector.tensor_tensor(
                    sc2, sc_sb.rearrange("p a s k -> p (a s k)"), cnorm4,
                    mybir.AluOpType.add,
                )
                sc3 = sc2.rearrange("p (a s k) -> p a s k", s=S, k=K)
                minv = small.tile([P, 4 * S], F32, tag="minv")
                nc.vector.tensor_reduce(
                    minv,
                    sc2.rearrange("p (g k) -> p g k", k=K),
                    axis=mybir.AxisListType.X,
                    op=mybir.AluOpType.min,
                )
                mask = small.tile([P, 4 * S * K], F32, tag="mask")
                nc.vector.tensor_tensor(
                    mask,
                    sc3,
                    minv.rearrange("p (a s) -> p a s", s=S)[:, :, :, None]
                        .to_broadcast((P, 4, S, K)),
                    mybir.AluOpType.is_equal,
                )
                t4 = small.tile([P, 4 * S * K], F32, tag="t4")
                nc.vector.tensor_tensor(
                    t4, mask, iotad8[:, : 4 * S * K], mybir.AluOpType.mult
                )
                dmax = small.tile([P, 4 * S], F32, tag="dmax")
                nc.vector.tensor_reduce(
                    dmax,
                    t4.rearrange("p (g k) -> p g k", k=K),
                    axis=mybir.AxisListType.X,
                    op=mybir.AluOpType.max,
                )
                # packed ids: 65535 - sum_s dmax * 16^s   (dmax = 15 - id)
                pk = small.tile([P, 4 * S], F32, tag="pk")
                nc.vector.tensor_tensor(pk, dmax, offs4, mybir.AluOpType.mult)
                pneg = small.tile([P, 4], F32, tag="pneg")
                nc.vector.tensor_reduce(
                    pneg,
                    pk.rearrange("p (a s) -> p a s", s=S),
                    axis=mybir.AxisListType.X,
                    op=mybir.AluOpType.add,
                )
                nc.vector.tensor_scalar(
                    idsf[:, 4 * m : 4 * m + 4], pneg, -1.0, 65535.0,
                    op0=mybir.AluOpType.mult, op1=mybir.AluOpType.add,
                )
                # clean one-hot (single 1 even on ties), duplicated x2 so the
                # z matmul can stack the hi/lo codebooks in the contraction:
                # onehotT2[t, (j, u, s, k)] = (iotad == dmax[j, s]), u = 0, 1
                onehotT2 = small.tile([P, 4, 2, S, K], BF16, tag="onehotT2")
                for u in range(2):
                    nc.vector.tensor_tensor(
                        onehotT2[:, :, u],
                        iotad8[:, : 4 * S * K].rearrange(
                            "p (a s k) -> p a s k", s=S, k=K
                        ),
                        dmax.rearrange("p (a s) -> p a s", s=S)[:, :, :, None]
                            .to_broadcast((P, 4, S, K)),
                        mybir.AluOpType.is_equal,
                    )
                # loss accumulation
                nc.vector.tensor_tensor(acc16, acc16, minv, mybir.AluOpType.add)

                # ---- gather z via one-hot matmul (bf16, exact to 2^-17) ----
                oh_ps = ps_oh.tile([2 * S * K, 4, P], BF16, tag="oh")
                for j in range(4):
                    nc.tensor.transpose(
                        oh_ps[:, j, :],
                        onehotT2[:, j],
                        identity_bf,
                    )
                oh_sb = small.tile([2 * S * K, 4, P], BF16, tag="ohsb")
                nc.scalar.copy(out=oh_sb, in_=oh_ps)
                z_mac = zsb_pool.tile([P, 4, D], F32, tag="zs")
                for j in range(4):
                    for half in (0, 1):
                        sl = slice(half * (D // 2), (half + 1) * (D // 2))
                        z_ps = ps_z.tile([P, D // 2], F32, tag="z")
                        nc.tensor.matmul(
                            z_ps, lhsT=oh_sb[:, j, :], rhs=cbd12[:, sl],
                            start=True, stop=True,
                        )
                        if (2 * j + half) % 4 != 3:
                            nc.scalar.copy(out=z_mac[:, j, sl], in_=z_ps)
                        else:
                            nc.vector.tensor_copy(out=z_mac[:, j, sl], in_=z_ps)
                pending_stores.append(
                    (
                        z_d[t0m : t0m + 4 * P, :].rearrange(
                            "(j p) dd -> p j dd", p=P
                        ),
                        z_mac,
                    )
                )

            for dst, src in pending_stores:
                nc.scalar.dma_start(out=dst, in_=src)
            pending_stores.clear()

            # ---- epilogue: ids out ----
            ids_ps = ps_raw.tile([nsub, P], F32, tag="raw")
            nc.tensor.transpose(ids_ps, idsf, identity)
            ids_int = small.tile([nsub, P], I32, tag="idsint")
            nc.vector.tensor_copy(out=ids_int, in_=ids_ps)
            nc.sync.dma_start(out=ids_d[:, :], in_=ids_int)

            # ---- epilogue: loss partial ----
            rt = small.tile([P, 1], F32, tag="rt")
            nc.vector.tensor_reduce(
                rt, acc16, axis=mybir.AxisListType.X, op=mybir.AluOpType.add
            )
            part_ps = ps_oh.tile([1, 1], F32, tag="oh")
            nc.tensor.matmul(part_ps, lhsT=ones, rhs=rt, start=True, stop=True)
            part_sb = small.tile([1, 1], F32, tag="partsb")
            nc.vector.tensor_copy(out=part_sb, in_=part_ps)
            nc.sync.dma_start(out=part_d[:, :], in_=part_sb)

    nc.finalize()
    return nc


def make_consts(cb: np.ndarray) -> dict[str, np.ndarray]:
    cb = np.ascontiguousarray(cb, dtype=np.float32)
    # cts[dp, c, k] = -2 * cb[c // 2, k, (c % 2) * 128 + dp], split hi/lo bf16
    cb_r = cb.reshape(S, K, 2, P)  # [s, k, half, dp]
    cts = np.ascontiguousarray(-2.0 * cb_r.transpose(3, 0, 2, 1).reshape(P, NCHUNK, K))
    cts1 = cts.astype(ml_dtypes.bfloat16)
    cts2 = (cts - cts1.astype(np.float32)).astype(ml_dtypes.bfloat16)
    ctsb = np.concatenate([cts1, cts2], axis=-1)  # [P, NCHUNK, 2K]
    cbd = np.zeros((S * K, D), dtype=np.float32)
    for s in range(S):
        cbd[s * K : (s + 1) * K, s * d : (s + 1) * d] = cb[s]
    cbd1 = cbd.astype(ml_dtypes.bfloat16)
    cbd2 = (cbd - cbd1.astype(np.float32)).astype(ml_dtypes.bfloat16)
    cbd12 = np.concatenate([cbd1, cbd2], axis=0)  # [2*S*K, D]
    cnorm1 = np.square(cb).sum(-1).reshape(-1)  # [S*K]
    cnorm4 = np.tile(cnorm1, (P, 4))
    iotad1 = np.tile((15.0 - np.arange(K)).astype(np.float32), 4 * S)
    iotad8 = np.tile(iotad1, (P, 2))
    offs1 = np.tile((16.0 ** np.arange(S)).astype(np.float32), 4)
    offs4 = np.tile(offs1, (P, 1))
    return {
        "ctsb": np.ascontiguousarray(ctsb),
        "cbd12": np.ascontiguousarray(cbd12),
        "cnorm4": cnorm4.astype(np.float32),
        "iotad8": iotad8.astype(np.float32),
        "offs4": offs4.astype(np.float32),
    }


def make_hbf(h: np.ndarray) -> np.ndarray:
    """Exact-to-2^-17 bf16 hi/lo re-encoding (h = a1 + a2), stored
    pre-transposed per 512-token macro tile:
    hbf[m, dp, u*8+c, t] = a_u[512m + t, 128c + dp]."""
    hf = np.ascontiguousarray(h, dtype=np.float32).reshape(-1, D)
    a1 = hf.astype(ml_dtypes.bfloat16)
    a2 = (hf - a1.astype(np.float32)).astype(ml_dtypes.bfloat16)
    A = np.stack([a1, a2], axis=1)          # [T, u, d]
    M = hf.shape[0] // (4 * P)
    A = A.reshape(M, 4 * P, 2, NCHUNK, P)   # [m, t, u, c, dp]
    X = A.transpose(0, 4, 2, 3, 1)          # [m, dp, u, c, t]
    return np.ascontiguousarray(X.reshape(M, P, 2 * NCHUNK, 4 * P))


_NC_CACHE: dict[int, bass.Bass] = {}
LAST_RESULTS = None


def _get_nc(t_core: int) -> bass.Bass:
    if t_core not in _NC_CACHE:
        _NC_CACHE[t_core] = build_nc(t_core)
    return _NC_CACHE[t_core]


def kernel(h: np.ndarray, codebooks: np.ndarray):
    h = np.ascontiguousarray(h, dtype=np.float32)
    cb = np.ascontiguousarray(codebooks, dtype=np.float32)
    t_core = (B * N) // NCORES
    nc = _get_nc(t_core)
    consts = make_consts(cb)
    hbf = make_hbf(h)
    hf64 = h.reshape(B * N, D)
    mac_core = t_core // (4 * P)
    in_maps = []
    for c in range(NCORES):
        m = {"hbf": hbf[c * mac_core : (c + 1) * mac_core]}
        m.update(consts)
        in_maps.append(m)
    global LAST_RESULTS
    kr = run_bass_kernel_spmd(nc, in_maps, list(range(NCORES)))
    LAST_RESULTS = kr
    res = kr.results
    z = np.concatenate([res[c]["z"] for c in range(NCORES)], axis=0).reshape(B, N, D)
    ids = np.concatenate(
        [res[c]["ids"].reshape(-1) for c in range(NCORES)]
    ).reshape(B, N).astype(np.int32)
    total = np.sum([np.float64(res[c]["partial"][0, 0]) for c in range(NCORES)])
    hsq = np.einsum("ij,ij->", hf64, hf64)
    vq_total = np.float32((1.0 + BETA) * (total + hsq) / (B * N * d))
    return z, ids, vq_total
